# revision 23
# baseline (speedup 1.0000x reference)
"""Complex Conv1D (VALID, stride 1) on Trainium2 — Bass/Tile, 8-core data-parallel.

Problem (hardcoded shapes):
  x_real/x_imag: [32, 4096, 64] f32, kernel_real/imag: [9, 64, 64] f32,
  bias_real/imag: [64] f32  ->  out [32, 4088, 64, 2] f32
  out_real = conv(xr, wr) - conv(xi, wi) + br
  out_imag = conv(xr, wi) + conv(xi, wr) + bi

Mapping: complex multiply as its 2x2 real block-matrix form so each tap is ONE
full 128-contract matmul:
  X_b [128, L]   rows 0:64 = xr[b].T (channels on partitions), 64:128 = xi[b].T
  W[k] [128,128] = [[wr[k], wi[k]], [-wi[k], wr[k]]]
  psum[128, T] += W[k].T @ X_b[:, l0+k : l0+k+T]   for k = 0..8
  psum rows 0:64 = real output (filters), rows 64:128 = imag output.
Batch is sharded 4-per-core across 8 cores; weights replicated. The kernel
emits the output transposed as [b, 128, L_out]; the host restores
[B, L_out, F, 2].
"""

import numpy as np

import concourse.bacc as bacc
import concourse.bass as bass
import concourse.mybir as mybir
from concourse.tile import TileContext
from concourse.bass_utils import run_bass_kernel_spmd

B, L, CIN, KT, F = 32, 4096, 64, 9, 64
LOUT = L - KT + 1  # 4088
NCORES = 8
BPC = B // NCORES  # batches per core
TL = 512  # output-tile width (one PSUM bank of fp32)
NLT = (LOUT + TL - 1) // TL  # 8

# Matmul operand dtype: float32r streams fp32 operands through the PE in a
# single reduced-precision pass (full rate for N>=256); plain float32 is the
# exact-but-4x-slower fallback; bfloat16 halves DMA traffic.
MM_DT_NAME = "bfloat16"


def _build_nc_v2(
    mm_dt,
    w_dt=None,
    xbufs=3,
    obufs=2,
    psbufs=4,
    warmup=24,
    evac="act",
    out_dt_name="float32",
    probe=None,
    out_ring="sp",
    out_chunks=2,
    x_chunks=2,
    loop_repeat=None,
):
    """v2: whole-batch chunked DMAs (x[b] in 2x512KB, out[b] in 2x1MB), x and
    out fully SBUF-resident per batch, bf16 operands, ACT-only PSUM evacs
    (DVE evacs measured slower), x-loads emitted ahead of out-stores in the
    SP HWDGE ring so the FIFO never parks a load behind a store."""
    nc = bacc.Bacc("TRN2", target_bir_lowering=False, debug=False, num_devices=NCORES)
    if w_dt is None:
        w_dt = mm_dt
    out_dt = getattr(mybir.dt, out_dt_name)

    x_d = nc.dram_tensor("x", [BPC, 128, L], mm_dt, kind="ExternalInput")
    w_d = nc.dram_tensor("w", [128, KT * 128], w_dt, kind="ExternalInput")
    bias_d = nc.dram_tensor("bias", [128, 1], mybir.dt.float32, kind="ExternalInput")
    out_d = nc.dram_tensor("out", [BPC, 128, LOUT], out_dt, kind="ExternalOutput")

    f32 = mybir.dt.float32
    ident = mybir.ActivationFunctionType.Identity

    with TileContext(nc) as tc:
        with (
            tc.tile_pool(name="wpool", bufs=1) as wpool,
            tc.tile_pool(name="xpool", bufs=xbufs) as xpool,
            tc.tile_pool(name="opool", bufs=obufs) as opool,
            tc.tile_pool(name="pspool", bufs=psbufs, space="PSUM") as pspool,
        ):
            wt = wpool.tile([128, KT * 128], w_dt)
            nc.sync.dma_start(wt[:], w_d[:])
            bias_t = wpool.tile([128, 1], f32)
            nc.sync.dma_start(bias_t[:], bias_d[:])

            if warmup:
                wps = pspool.tile([128, 128], f32, tag="wps", bufs=1)
                for i in range(warmup):
                    nc.tensor.matmul(
                        wps[:], wt[:, 0:128], wt[:, 0:128],
                        start=True, stop=True, skip_group_check=True,
                    )

            import contextlib

            loop_cm = (
                tc.For_i(0, loop_repeat, 1)
                if loop_repeat is not None
                else contextlib.nullcontext()
            )
            n_evac = 0
            if probe == "nodma":
                # perf probe: load x once outside the loop; tiny out DMA.
                xts = []
                for b in range(BPC):
                    xtp = xpool.tile([128, L], mm_dt, tag=f"xp{b}", name=f"xp{b}",
                                     bufs=1)
                    nc.sync.dma_start(xtp[:], x_d[b])
                    xts.append(xtp)

            def load_x(b):
                xt = xpool.tile([128, L], mm_dt, tag="xt", name=f"xt{b}")
                xc = (L + x_chunks - 1) // x_chunks
                for c in range(x_chunks):
                    c0, c1 = c * xc, min(L, (c + 1) * xc)
                    nc.sync.dma_start(xt[:, c0:c1], x_d[b, :, c0:c1])
                return xt

            with loop_cm:
                # Prefetch ordering: emit batch b+1's x-load DMA before batch
                # b's out-store so the SP HWDGE ring (FIFO) never parks an
                # x-load behind a 2MB store the PE is not waiting for.
                xq = {}
                if probe != "nodma":
                    xq[0] = load_x(0)
                    if BPC > 1 and xbufs > 1:
                        xq[1] = load_x(1)
                for b in range(BPC):
                    if probe == "nodma":
                        xt = xts[b]
                    else:
                        xt = xq.pop(b) if b in xq else load_x(b)
                    ot = opool.tile([128, LOUT], out_dt, tag="ot")
                    for j in range(NLT):
                        l0 = j * TL
                        t = min(TL, LOUT - l0)
                        ps = pspool.tile([128, TL], f32, tag="ps")
                        for k in range(KT):
                            nc.tensor.matmul(
                                ps[:, :t],
                                wt[:, k * 128 : (k + 1) * 128],
                                xt[:, l0 + k : l0 + k + t],
                                start=(k == 0),
                                stop=(k == KT - 1),
                            )
                        if evac == "tiny":
                            nc.vector.tensor_copy(ot[:, l0 : l0 + 1], ps[:, 0:1])
                        elif evac == "alt" and n_evac % 2 == 1:
                            nc.vector.tensor_scalar_add(
                                ot[:, l0 : l0 + t], ps[:, :t], bias_t[:]
                            )
                        else:
                            nc.scalar.activation(
                                ot[:, l0 : l0 + t], ps[:, :t], ident, bias=bias_t[:]
                            )
                        n_evac += 1
                    if probe == "nodma":
                        nc.sync.dma_start(out_d[b, :, 0:1], ot[:, 0:1])
                    else:
                        nb = b + 2
                        if nb < BPC and nb not in xq:
                            xq[nb] = load_x(nb)
                        out_eng = nc.scalar if out_ring == "act" else nc.sync
                        oc = (LOUT + out_chunks - 1) // out_chunks
                        for c in range(out_chunks):
                            c0, c1 = c * oc, min(LOUT, (c + 1) * oc)
                            out_eng.dma_start(out_d[b, :, c0:c1], ot[:, c0:c1])

    nc.compile()
    return nc



def _build_nc_v7(
    mm_dt,
    w_dt=None,
    xbufs=3,
    obufs=2,
    warmup=24,
    out_chunks=2,
    x_chunks=2,
    loop_repeat=None,
):
    """v7: pair-interleaved taps over only 4 PSUM banks. Each pair of output
    tiles accumulates in 2 banks with taps outer (MMs alternate banks every
    instruction, so the PE never micro-idles at a group boundary and HAM
    stays warm); pair-slots ping-pong across 2x2 banks. ACT-only evacs,
    v6-style DMA prefetch ordering."""
    nc = bacc.Bacc("TRN2", target_bir_lowering=False, debug=False, num_devices=NCORES)
    if w_dt is None:
        w_dt = mm_dt

    x_d = nc.dram_tensor("x", [BPC, 128, L], mm_dt, kind="ExternalInput")
    w_d = nc.dram_tensor("w", [128, KT * 128], w_dt, kind="ExternalInput")
    bias_d = nc.dram_tensor("bias", [128, 1], mybir.dt.float32, kind="ExternalInput")
    out_d = nc.dram_tensor("out", [BPC, 128, LOUT], mybir.dt.float32, kind="ExternalOutput")

    f32 = mybir.dt.float32
    ident = mybir.ActivationFunctionType.Identity

    with TileContext(nc) as tc:
        with (
            tc.tile_pool(name="wpool", bufs=1) as wpool,
            tc.tile_pool(name="xpool", bufs=xbufs) as xpool,
            tc.tile_pool(name="opool", bufs=obufs) as opool,
            tc.tile_pool(name="pspool", bufs=2, space="PSUM") as pspool,
        ):
            wt = wpool.tile([128, KT * 128], w_dt)
            nc.sync.dma_start(wt[:], w_d[:])
            bias_t = wpool.tile([128, 1], f32)
            nc.sync.dma_start(bias_t[:], bias_d[:])

            if warmup:
                wps = pspool.tile([128, 128], f32, tag="wps", bufs=1)
                for i in range(warmup):
                    nc.tensor.matmul(
                        wps[:], wt[:, 0:128], wt[:, 0:128],
                        start=True, stop=True, skip_group_check=True,
                    )

            import contextlib

            loop_cm = (
                tc.For_i(0, loop_repeat, 1)
                if loop_repeat is not None
                else contextlib.nullcontext()
            )

            def load_x(b):
                xt = xpool.tile([128, L], mm_dt, tag="xt", name=f"xt{b}")
                xc = (L + x_chunks - 1) // x_chunks
                for c in range(x_chunks):
                    c0, c1 = c * xc, min(L, (c + 1) * xc)
                    nc.sync.dma_start(xt[:, c0:c1], x_d[b, :, c0:c1])
                return xt

            with loop_cm:
                xq = {0: load_x(0)}
                if BPC > 1 and xbufs > 1:
                    xq[1] = load_x(1)
                for b in range(BPC):
                    xt = xq.pop(b) if b in xq else load_x(b)
                    ot = opool.tile([128, LOUT], f32, tag="ot")
                    for p0 in range(0, NLT, 2):
                        psa = pspool.tile([128, TL], f32, tag="psa", name="psa")
                        psb = pspool.tile([128, TL], f32, tag="psb", name="psb")
                        for k in range(KT):
                            for j, ps in ((p0, psa), (p0 + 1, psb)):
                                l0 = j * TL
                                t = min(TL, LOUT - l0)
                                nc.tensor.matmul(
                                    ps[:, :t],
                                    wt[:, k * 128 : (k + 1) * 128],
                                    xt[:, l0 + k : l0 + k + t],
                                    start=(k == 0),
                                    stop=(k == KT - 1),
                                    skip_group_check=True,
                                )
                        for j, ps in ((p0, psa), (p0 + 1, psb)):
                            l0 = j * TL
                            t = min(TL, LOUT - l0)
                            nc.scalar.activation(
                                ot[:, l0 : l0 + t], ps[:, :t], ident,
                                bias=bias_t[:],
                            )
                    nb = b + 2
                    if nb < BPC and nb not in xq:
                        xq[nb] = load_x(nb)
                    oc = (LOUT + out_chunks - 1) // out_chunks
                    for c in range(out_chunks):
                        c0, c1 = c * oc, min(LOUT, (c + 1) * oc)
                        nc.sync.dma_start(out_d[b, :, c0:c1], ot[:, c0:c1])

    nc.compile()
    return nc


def _build_nc_v3(
    mm_dt,
    w_dt=None,
    xbufs=2,
    obufs=2,
    evac="alt",
    out_chunks=2,
    x_chunks=2,
    loop_repeat=None,
):
    """v3: tap-outer sweep. Per batch, hold all 8 output tiles in the 8 PSUM
    banks; for each of the 9 taps, load the tap weight once and sweep all 8
    tiles with it (9 LDWEIGHTS per batch instead of 72, and constant rhs
    alignment class per sweep). Evac banks after the last tap, alternating
    scalar/vector engines; chunked whole-batch DMAs."""
    nc = bacc.Bacc("TRN2", target_bir_lowering=False, debug=False, num_devices=NCORES)
    if w_dt is None:
        w_dt = mm_dt

    x_d = nc.dram_tensor("x", [BPC, 128, L], mm_dt, kind="ExternalInput")
    w_d = nc.dram_tensor("w", [128, KT * 128], w_dt, kind="ExternalInput")
    bias_d = nc.dram_tensor("bias", [128, 1], mybir.dt.float32, kind="ExternalInput")
    out_d = nc.dram_tensor("out", [BPC, 128, LOUT], mybir.dt.float32, kind="ExternalOutput")

    f32 = mybir.dt.float32
    ident = mybir.ActivationFunctionType.Identity

    with TileContext(nc) as tc:
        with (
            tc.tile_pool(name="wpool", bufs=1) as wpool,
            tc.tile_pool(name="xpool", bufs=xbufs) as xpool,
            tc.tile_pool(name="opool", bufs=obufs) as opool,
            tc.tile_pool(name="pspool", bufs=1, space="PSUM") as pspool,
        ):
            wt = wpool.tile([128, KT * 128], w_dt)
            nc.sync.dma_start(wt[:], w_d[:])
            bias_t = wpool.tile([128, 1], f32)
            nc.sync.dma_start(bias_t[:], bias_d[:])

            import contextlib

            loop_cm = (
                tc.For_i(0, loop_repeat, 1)
                if loop_repeat is not None
                else contextlib.nullcontext()
            )
            with loop_cm:
                for b in range(BPC):
                    xt = xpool.tile([128, L], mm_dt, tag="xt")
                    xc = (L + x_chunks - 1) // x_chunks
                    for c in range(x_chunks):
                        c0 = c * xc
                        c1 = min(L, c0 + xc)
                        nc.sync.dma_start(xt[:, c0:c1], x_d[b, :, c0:c1])
                    ot = opool.tile([128, LOUT], f32, tag="ot")
                    pss = [
                        pspool.tile([128, TL], f32, tag=f"ps{j}", name=f"ps{j}")
                        for j in range(NLT)
                    ]
                    for k in range(KT):
                        for j in range(NLT):
                            l0 = j * TL
                            t = min(TL, LOUT - l0)
                            nc.tensor.matmul(
                                pss[j][:, :t],
                                wt[:, k * 128 : (k + 1) * 128],
                                xt[:, l0 + k : l0 + k + t],
                                start=(k == 0),
                                stop=(k == KT - 1),
                                skip_group_check=True,
                            )
                    for j in range(NLT):
                        l0 = j * TL
                        t = min(TL, LOUT - l0)
                        if evac == "alt" and j % 2 == 1:
                            nc.vector.tensor_scalar_add(
                                ot[:, l0 : l0 + t], pss[j][:, :t], bias_t[:]
                            )
                        else:
                            nc.scalar.activation(
                                ot[:, l0 : l0 + t], pss[j][:, :t], ident,
                                bias=bias_t[:],
                            )
                    oc = (LOUT + out_chunks - 1) // out_chunks
                    for c in range(out_chunks):
                        c0 = c * oc
                        c1 = min(LOUT, c0 + oc)
                        nc.sync.dma_start(out_d[b, :, c0:c1], ot[:, c0:c1])

    nc.compile()
    return nc


def _build_nc_v4(
    mm_dt,
    w_dt=None,
    xbufs=2,
    obufs=2,
    evac="alt",
    out_chunks=2,
    x_chunks=2,
    pair=2,
    loop_repeat=None,
):
    """v4: tap-outer over PAIRS of output tiles. For each pair of PSUM banks,
    sweep taps k=0..8 issuing 2 matmuls per tap (LDWEIGHTS amortized 2x, only
    2-deep PSUM bank rotation — measured full-rate 164ns/MM vs 244 for
    tap-inner and 264+ for 8-deep rotation). Evacs alternate ACT/DVE and
    overlap the next pair's matmuls; whole-batch chunked DMAs."""
    nc = bacc.Bacc("TRN2", target_bir_lowering=False, debug=False, num_devices=NCORES)
    if w_dt is None:
        w_dt = mm_dt

    x_d = nc.dram_tensor("x", [BPC, 128, L], mm_dt, kind="ExternalInput")
    w_d = nc.dram_tensor("w", [128, KT * 128], w_dt, kind="ExternalInput")
    bias_d = nc.dram_tensor("bias", [128, 1], mybir.dt.float32, kind="ExternalInput")
    out_d = nc.dram_tensor("out", [BPC, 128, LOUT], mybir.dt.float32, kind="ExternalOutput")

    f32 = mybir.dt.float32
    ident = mybir.ActivationFunctionType.Identity

    with TileContext(nc) as tc:
        with (
            tc.tile_pool(name="wpool", bufs=1) as wpool,
            tc.tile_pool(name="xpool", bufs=xbufs) as xpool,
            tc.tile_pool(name="opool", bufs=obufs) as opool,
            tc.tile_pool(name="pspool", bufs=1, space="PSUM") as pspool,
        ):
            wt = wpool.tile([128, KT * 128], w_dt)
            nc.sync.dma_start(wt[:], w_d[:])
            bias_t = wpool.tile([128, 1], f32)
            nc.sync.dma_start(bias_t[:], bias_d[:])

            import contextlib

            loop_cm = (
                tc.For_i(0, loop_repeat, 1)
                if loop_repeat is not None
                else contextlib.nullcontext()
            )
            with loop_cm:
                for b in range(BPC):
                    xt = xpool.tile([128, L], mm_dt, tag="xt")
                    xc = (L + x_chunks - 1) // x_chunks
                    for c in range(x_chunks):
                        c0 = c * xc
                        c1 = min(L, c0 + xc)
                        nc.sync.dma_start(xt[:, c0:c1], x_d[b, :, c0:c1])
                    ot = opool.tile([128, LOUT], f32, tag="ot")
                    pss = [
                        pspool.tile([128, TL], f32, tag=f"ps{j}", name=f"ps{j}")
                        for j in range(NLT)
                    ]
                    for p0 in range(0, NLT, pair):
                        js = range(p0, min(p0 + pair, NLT))
                        for k in range(KT):
                            for j in js:
                                l0 = j * TL
                                t = min(TL, LOUT - l0)
                                nc.tensor.matmul(
                                    pss[j][:, :t],
                                    wt[:, k * 128 : (k + 1) * 128],
                                    xt[:, l0 + k : l0 + k + t],
                                    start=(k == 0),
                                    stop=(k == KT - 1),
                                    skip_group_check=True,
                                )
                        for j in js:
                            l0 = j * TL
                            t = min(TL, LOUT - l0)
                            if evac == "alt" and j % 2 == 1:
                                nc.vector.tensor_scalar_add(
                                    ot[:, l0 : l0 + t], pss[j][:, :t], bias_t[:]
                                )
                            else:
                                nc.scalar.activation(
                                    ot[:, l0 : l0 + t], pss[j][:, :t], ident,
                                    bias=bias_t[:],
                                )
                    oc = (LOUT + out_chunks - 1) // out_chunks
                    for c in range(out_chunks):
                        c0 = c * oc
                        c1 = min(LOUT, c0 + oc)
                        nc.sync.dma_start(out_d[b, :, c0:c1], ot[:, c0:c1])

    nc.compile()
    return nc


def _build_nc(
    mm_dt,
    w_dt=None,
    xbufs=3,
    obufs=4,
    psbufs=4,
    warmup=0,
    evac="act",
    repeat=1,
    loop_repeat=None,
):
    nc = bacc.Bacc("TRN2", target_bir_lowering=False, debug=False, num_devices=NCORES)
    if w_dt is None:
        w_dt = mm_dt

    x_d = nc.dram_tensor("x", [BPC, 128, L], mm_dt, kind="ExternalInput")
    w_d = nc.dram_tensor("w", [128, KT * 128], w_dt, kind="ExternalInput")
    bias_d = nc.dram_tensor("bias", [128, 1], mybir.dt.float32, kind="ExternalInput")
    out_d = nc.dram_tensor("out", [BPC, 128, LOUT], mybir.dt.float32, kind="ExternalOutput")

    f32 = mybir.dt.float32
    ident = mybir.ActivationFunctionType.Identity

    with TileContext(nc) as tc:
        with (
            tc.tile_pool(name="wpool", bufs=1) as wpool,
            tc.tile_pool(name="xpool", bufs=xbufs) as xpool,
            tc.tile_pool(name="opool", bufs=obufs) as opool,
            tc.tile_pool(name="pspool", bufs=psbufs, space="PSUM") as pspool,
        ):
            wt = wpool.tile([128, KT * 128], w_dt)
            nc.sync.dma_start(wt[:], w_d[:])
            bias_t = wpool.tile([128, 1], f32)
            nc.sync.dma_start(bias_t[:], bias_d[:])

            if warmup:
                # Keep the PE busy (HAM ramp) while the first x tiles load.
                wps = pspool.tile([128, 128], f32, tag="wps", bufs=1)
                for i in range(warmup):
                    nc.tensor.matmul(
                        wps[:], wt[:, 0:128], wt[:, 0:128],
                        start=True, stop=True, skip_group_check=True,
                    )

            import contextlib

            loop_cm = (
                tc.For_i(0, loop_repeat, 1)
                if loop_repeat is not None
                else contextlib.nullcontext()
            )
            n_evac = 0
            with loop_cm:
              for _rep in range(repeat):
                for b in range(BPC):
                    for j in range(NLT):
                        l0 = j * TL
                        t = min(TL, LOUT - l0)
                        w_in = min(L, l0 + t + KT - 1) - l0
                        xt = xpool.tile([128, TL + KT - 1], mm_dt, tag="xt")
                        nc.sync.dma_start(xt[:, :w_in], x_d[b, :, l0 : l0 + w_in])
                        ps = pspool.tile([128, TL], f32, tag="ps")
                        for k in range(KT):
                            nc.tensor.matmul(
                                ps[:, :t],
                                wt[:, k * 128 : (k + 1) * 128],
                                xt[:, k : k + t],
                                start=(k == 0),
                                stop=(k == KT - 1),
                            )
                        ot = opool.tile([128, TL], f32, tag="ot")
                        if evac == "alt" and n_evac % 2 == 1:
                            nc.vector.tensor_scalar_add(
                                ot[:, :t], ps[:, :t], bias_t[:]
                            )
                        else:
                            nc.scalar.activation(
                                ot[:, :t], ps[:, :t], ident, bias=bias_t[:]
                            )
                        n_evac += 1
                        nc.sync.dma_start(out_d[b, :, l0 : l0 + t], ot[:, :t])

    nc.compile()
    return nc


def _pack(x_real, x_imag, kernel_real, kernel_imag, bias_real, bias_imag, np_dt,
          w_np_dt=None):
    if w_np_dt is None:
        w_np_dt = np_dt
    X = np.empty((B, 128, L), np_dt)
    X[:, :CIN] = x_real.transpose(0, 2, 1)
    X[:, CIN:] = x_imag.transpose(0, 2, 1)
    Wk = np.empty((KT, 128, 128), np.float32)
    Wk[:, :CIN, :F] = kernel_real
    Wk[:, :CIN, F:] = kernel_imag
    Wk[:, CIN:, :F] = -kernel_imag
    Wk[:, CIN:, F:] = kernel_real
    W2 = Wk.transpose(1, 0, 2).reshape(128, KT * 128).astype(w_np_dt)
    bias2 = (
        np.concatenate([bias_real, bias_imag]).reshape(128, 1).astype(np.float32)
    )
    return X, np.ascontiguousarray(W2), bias2


def _parse_dt(name):
    name = name or MM_DT_NAME
    if "," in name:
        xn, wn = name.split(",")
    else:
        xn = wn = name
    return getattr(mybir.dt, xn), getattr(mybir.dt, wn)


def _prepare(inputs, mm_dt_name=None, build_kw=None):
    mm_dt, w_dt = _parse_dt(mm_dt_name)
    np_dt = mybir.dt.np(mm_dt)
    w_np_dt = mybir.dt.np(w_dt)
    args = {
        k: np.asarray(inputs[k], np.float32)
        for k in (
            "x_real", "x_imag", "kernel_real", "kernel_imag", "bias_real", "bias_imag",
        )
    }
    X, W2, bias2 = _pack(np_dt=np_dt, w_np_dt=w_np_dt, **args)

    build_kw = dict(build_kw or {})
    ver = build_kw.pop("ver", 2)
    builder = {1: _build_nc, 2: _build_nc_v2, 3: _build_nc_v3, 4: _build_nc_v4, 7: _build_nc_v7}[ver]
    nc = builder(mm_dt, w_dt=w_dt, **build_kw)
    in_maps = [
        {
            "x": np.ascontiguousarray(X[i * BPC : (i + 1) * BPC]),
            "w": W2,
            "bias": bias2,
        }
        for i in range(NCORES)
    ]
    return nc, in_maps


def _gather(results):
    O = np.concatenate([r["out"] for r in results], axis=0)  # [32, 128, 4088]
    O = O.reshape(B, 2, F, LOUT).transpose(0, 3, 2, 1)  # [B, LOUT, F, 2]
    return np.ascontiguousarray(O, dtype=np.float32)


def _run(inputs, trace=False, mm_dt_name=None):
    nc, in_maps = _prepare(inputs, mm_dt_name)
    res = run_bass_kernel_spmd(nc, in_maps, core_ids=list(range(NCORES)), trace=trace)
    return _gather(res.results), res


def kernel(**inputs) -> np.ndarray:
    out, _ = _run(inputs, trace=False)
    return out

